# revision 19
# baseline (speedup 1.0000x reference)
"""
Trainium2 Bass kernel for nn_CIN (xDeepFM-style Compressed Interaction Network).

Reference computation (B=1024, F=39, D=32):
    x0 = x;  h = x
    for layer i in 0..2:
        z[b,d,:] = outer(x0[b,:,d], h[b,:,d]).flatten()     # (B, D, F*Hp)
        out = relu(z @ W_i + b_i)                           # (B, D, 256) -> (B, 256, D)
        h = out[:, :128]; finals.append(out[:, 128:])       # (last layer: all 256)
    res = concat(finals, 1).sum(-1) @ fc_w + fc_b           # (B, 1)

Strategy (data-parallel over 8 cores, 128 samples each):
  Everything on-chip lives TRANSPOSED: activations as [channel, (b,d)] so that
  - the matmul contraction (f*h) is on the partition axis (weights stationary),
  - the per-sample Khatri-Rao z-formation  z^T[f*128+h, bd] = x0^T[f,bd]*h^T[h,bd]
    is a plain VectorEngine tensor_tensor multiply against a host-precomputed
    128-partition broadcast of x0^T (per f),
  - bias+relu is a per-partition ScalarEngine activation (o on partitions),
  - the final fc_w contraction is a TensorE matvec over partitions.
  Layer-0's z is a pure function of the input x, so it is precomputed on host
  (Khatri-Rao of x0 with itself, ~0.03% of total FLOPs) and streamed.
  Layers 1 and 2 are processed per 512-wide bd-chunk back to back so the
  broadcast-x0 tiles are loaded once per chunk and reused by both layers.
  All matmul inputs are bf16 (fp32 PSUM accumulation).
"""

import os
import sys

import numpy as np

for _p in ("/opt/trn_rl_repo",):
    if os.path.isdir(_p) and _p not in sys.path:
        sys.path.append(_p)

import ml_dtypes

import concourse.bass as bass
import concourse.mybir as mybir
import concourse.tile as tile
from concourse import bacc
from concourse.bass_utils import run_bass_kernel_spmd

BF16 = ml_dtypes.bfloat16

# Problem constants (hardcoded per contract).
B, F, D = 1024, 39, 32
O = 256            # per-layer conv output channels
NCORES = 8
BC = B // NCORES   # samples per core = 128
NBD = BC * D       # bd columns per core = 4096
CH = 512           # free-dim chunk width
NCH = NBD // CH    # 8 chunks
NP0 = F * (F + 1) // 2   # layer-0 folded symmetric pairs = 780
K0C = 7                  # layer-0 K chunks (780 padded to 896)
K0 = K0C * 128
FBLK = 8           # f's per x0-broadcast DMA block
FBLKS = [8, 8, 8, 8, 7]

LAST_RESULT = None  # BassKernelResults of the most recent run (for test.py)
_CACHE = {}


def _build_program():
    """Build + compile the per-core Bass/Tile program (identical on all cores)."""
    nc = bacc.Bacc("TRN2", target_bir_lowering=False, debug=False)
    dt = mybir.dt

    z0t = nc.dram_tensor("z0t", [NCH, 128, K0C, CH], dt.bfloat16,
                         kind="ExternalInput").ap()
    # broadcast-x0, pre-arranged per (bd-chunk, f-block) so every DMA reads
    # partition-contiguous 8KB runs (descriptor-light on the HWDGE ring)
    x0b = nc.dram_tensor("x0b", [NCH, len(FBLKS), 128, FBLK * CH], dt.bfloat16,
                         kind="ExternalInput").ap()
    w0 = nc.dram_tensor("w0", [128, K0C * O], dt.bfloat16, kind="ExternalInput").ap()
    w1 = nc.dram_tensor("w1", [128, F * O], dt.bfloat16, kind="ExternalInput").ap()
    w2 = nc.dram_tensor("w2", [128, F * O], dt.bfloat16, kind="ExternalInput").ap()
    bia = nc.dram_tensor("bia", [128, 6], dt.float32, kind="ExternalInput").ap()
    fcw = nc.dram_tensor("fcw", [128, 4], dt.bfloat16, kind="ExternalInput").ap()
    acc = nc.dram_tensor("acc", [NCH, 16], dt.float32, kind="ExternalOutput").ap()

    with tile.TileContext(nc) as tc:
        with (
            tc.tile_pool(name="consts", bufs=1) as consts,
            tc.tile_pool(name="state", bufs=1) as state,
            tc.tile_pool(name="z0p", bufs=K0C) as z0p,
            tc.tile_pool(name="xbp", bufs=7) as xbp,
            tc.tile_pool(name="zp", bufs=3) as zp,
            tc.tile_pool(name="accp", bufs=2) as accp,
            tc.tile_pool(name="f23p", bufs=4) as f23p,
            tc.tile_pool(name="redp", bufs=2) as redp,
            tc.tile_pool(name="psp", bufs=8, space="PSUM") as psp,
        ):
            w0_sb = consts.tile([128, K0C * O], dt.bfloat16)
            w1_sb = consts.tile([128, F * O], dt.bfloat16)
            w2_sb = consts.tile([128, F * O], dt.bfloat16)
            bia_sb = consts.tile([128, 6], dt.float32)
            fcw_sb = consts.tile([128, 4], dt.bfloat16)

            # HAM warm-up: garbage matmuls issued before any DMA dependency so
            # the PE's activity window flips to full clock while the ~7us NEFF
            # preamble + first input DMAs are still in flight.
            warm = consts.tile([128, CH], dt.bfloat16)
            nc.vector.memset(warm[:], 0)
            warm_ps = psp.tile([128, CH], dt.float32, name="warm_ps", tag="ps")
            for _ in range(12):
                nc.tensor.matmul(warm_ps[:], lhsT=warm[:, 0:128], rhs=warm[:],
                                 start=True, stop=True)

            def wblock(w_sb, w_dram, blk):
                f0 = sum(FBLKS[:blk])
                w = FBLKS[blk]
                nc.sync.dma_start(w_sb[:, f0 * O:(f0 + w) * O],
                                  w_dram[:, f0 * O:(f0 + w) * O])

            h1 = state.tile([128, NBD], dt.bfloat16)   # layer-1 carried state^T
            h2 = state.tile([128, NBD], dt.bfloat16)   # layer-2 carried state^T
            fin = state.tile([128, 2 * NBD], dt.bfloat16)  # finals^T of layers 0,1

            def epilogue(ps, bias_col, dst):
                # dst = bf16(relu(psum + bias)), bias per-partition
                nc.scalar.activation(
                    dst, ps[:], mybir.ActivationFunctionType.Relu,
                    bias=bia_sb[:, bias_col:bias_col + 1], scale=1.0,
                )

            # ---- Layer 0, column-chunked -------------------------------------
            # z0 is the symmetric Khatri-Rao of x0 with itself, host-folded
            # over (f,h)<->(h,f) so K shrinks 1521 -> 780. It ships per
            # bd-chunk as [128p, 7k, 512] (1MB just-in-time DMAs) and L0-(c+1)
            # runs inside chunk c's pipeline as the PE gap-filler while the
            # relu(h2) -> z2 chain resolves — so the DMA head shrinks from
            # ~13MB to ~3MB before the first real matmul.
            K6 = NP0 - 6 * 128  # real contraction rows in the last k-chunk (12)

            def load_z0cc(c):
                z0_t = z0p.tile([128, K0C, CH], dt.bfloat16, name=f"z0_{c}",
                                tag="z0")
                nc.sync.dma_start(z0_t[:, 0:K0C - 1, :], z0t[c][:, 0:K0C - 1, :])
                nc.sync.dma_start(z0_t[0:K6, K0C - 1, :],
                                  z0t[c][0:K6, K0C - 1, :])
                return z0_t

            def l0(c, z0_t):
                ps0 = new_ps(f"ps0_{c}")
                for k in range(K0C):
                    np_k = K6 if k == K0C - 1 else 128
                    for hf in range(2):
                        nc.tensor.matmul(
                            ps0[hf][:],
                            lhsT=w0_sb[0:np_k,
                                       k * O + hf * 128: k * O + hf * 128 + 128],
                            rhs=z0_t[0:np_k, k, :],
                            start=(k == 0), stop=(k == K0C - 1),
                        )
                epilogue(ps0[0], 0, h1[:, c * CH: (c + 1) * CH])
                epilogue(ps0[1], 1,
                         fin[:, 0 * NBD + c * CH: 0 * NBD + (c + 1) * CH])

            # ---- Layers 1+2 fused per bd-chunk ------------------------------
            def blocks(blk_list):
                # (blk, j0, nf, f0) subgroups: 4-f DVE batches within f-blocks
                out = []
                for blk in blk_list:
                    bw = FBLKS[blk]
                    for j0, nf in ((0, 4), (4, bw - 4)):
                        out.append((blk, j0, nf, blk * FBLK + j0))
                return out

            def ll_part(li, c, ps, xbts, h_in, blk_list):
                # TT z-formation + matmuls of layer li+1 for chunk c, f-blocks
                # blk_list. One DVE op forms z^T for nf f's; the h-chunk
                # operand is re-read via a stride-0 AP dim.
                w_sb = (w1_sb, w2_sb)[li]
                csl = slice(c * CH, (c + 1) * CH)
                for blk, j0, nf, f0 in blocks(blk_list):
                    zt = zp.tile([128, 4 * CH], dt.bfloat16, tag="zt")
                    nc.vector.tensor_mul(
                        zt[:].rearrange("p (f c) -> p f c", f=4)[:, :nf],
                        xbts[blk][:, j0 * CH: (j0 + nf) * CH]
                            .rearrange("p (f c) -> p f c", f=nf),
                        h_in[:, csl].unsqueeze(1).broadcast_to((128, nf, CH)),
                    )
                    for i in range(nf):
                        f = f0 + i
                        for hf in range(2):
                            nc.tensor.matmul(
                                ps[hf][:],
                                lhsT=w_sb[:, f * O + hf * 128: f * O + hf * 128 + 128],
                                rhs=zt[:, i * CH: (i + 1) * CH],
                                start=(f == 0), stop=(f == F - 1),
                            )

            def load_xb(c, bi, eng=None):
                # two pieces aligned with ll_part's (0,4)/(4,..) DVE subgroups
                # so the first z-formation can start when half the block lands
                w = FBLKS[bi]
                xbt = xbp.tile([128, w * CH], dt.bfloat16, tag="xbt",
                               name=f"xbt_{c}_{bi}")
                e = eng or nc.sync
                e.dma_start(xbt[:, 0:4 * CH], x0b[c, bi, :, 0:4 * CH])
                e.dma_start(xbt[:, 4 * CH:w * CH], x0b[c, bi, :, 4 * CH:w * CH])
                return xbt

            def new_ps(nm):
                return [psp.tile([128, CH], dt.float32, name=f"{nm}_{hf}", tag="ps")
                        for hf in range(2)]

            def do_mv(c, f23):
                # d-sum each of the 4 finals on the DVE (columns are b-major,
                # d-minor: 16 groups of 32), then a tiny N=16 fc contraction
                # over channel partitions. Moves ~7us of N=512 matvecs off the
                # PE and shortens the serial tail after the last chunk.
                red = redp.tile([128, 4, 16], dt.bfloat16, tag="red",
                                name=f"red_{c}")
                fin4 = (fin[:].rearrange("p (j n) -> p j n", j=2)
                        [:, :, c * CH:(c + 1) * CH]
                        .rearrange("p j (b d) -> p j b d", b=16))
                with nc.allow_low_precision(
                        reason="DVE accumulates fp32 internally; bf16 is only "
                               "the stored d-sum, matching the baseline's "
                               "bf16-finals matvec precision"):
                    nc.vector.reduce_sum(red[:, 0:2, :].unsqueeze(3), fin4,
                                         axis=mybir.AxisListType.X)
                    for hf in range(2):
                        nc.vector.reduce_sum(
                            red[:, 2 + hf, :].unsqueeze(2),
                            f23[hf][:].rearrange("p (b d) -> p b d", b=16),
                            axis=mybir.AxisListType.X)
                mv = psp.tile([1, 16], dt.float32, name=f"mv_{c}", tag="ps")
                for j in range(4):
                    nc.tensor.matmul(
                        mv[:], lhsT=fcw_sb[:, j:j + 1], rhs=red[:, j, :],
                        start=(j == 0), stop=(j == 3),
                    )
                acct = accp.tile([1, 16], dt.float32, tag="acct",
                                 name=f"acct_{c}")
                nc.scalar.copy(acct[:], mv[:])
                nc.sync.dma_start(acc[c:c + 1, :], acct[:])

            # Software-pipelined main loop. PE stream per chunk c:
            #   [L1-c rest] [L0-(c+1)] [mv c-1] [L2-c blk0] [L1-head c+1]
            #   [L2-c rest]
            # L0-(c+1) has no on-chip dependency chain, so it fills the PE
            # while chunk c's relu(h2) -> z2 DVE chain resolves; by the time
            # L1-head-(c+1) issues, h1[c+1] from L0-(c+1)'s epilogue is ready.
            # head DMAs split across both rings (each gets ~half the HBM
            # share), ordered by first-use time:
            #   sync:   z0cc0 | bia | xb(0,1..4) | w2b0 | z0cc1 | fcw
            #   scalar: w0 | xb(0,0) | w1b0
            z0cc = load_z0cc(0)
            nc.sync.dma_start(bia_sb[:], bia)
            nc.scalar.dma_start(w0_sb[:], w0)
            xbts_c = [load_xb(0, 0, eng=nc.scalar)]
            nc.scalar.dma_start(w1_sb[:, 0:FBLKS[0] * O], w1[:, 0:FBLKS[0] * O])
            xbts_c += [load_xb(0, bi) for bi in range(1, len(FBLKS))]
            wblock(w2_sb, w2, 0)
            z0cc_n = load_z0cc(1)
            nc.sync.dma_start(fcw_sb[:], fcw)
            l0(0, z0cc)
            ps1 = new_ps("ps1_0")
            ll_part(0, 0, ps1, xbts_c, h1, [0])
            # bulk weights ride the Scalar ring; issued after l0(0)'s
            # epilogues so their descriptor-gen can't delay the first relus
            for blk in range(1, len(FBLKS)):
                f0 = sum(FBLKS[:blk])
                bw = FBLKS[blk]
                nc.scalar.dma_start(w1_sb[:, f0 * O:(f0 + bw) * O],
                                    w1[:, f0 * O:(f0 + bw) * O])
                nc.scalar.dma_start(w2_sb[:, f0 * O:(f0 + bw) * O],
                                    w2[:, f0 * O:(f0 + bw) * O])
            prev_f23 = None
            for c in range(8):
                csl = slice(c * CH, (c + 1) * CH)
                # rest of layer 1 for chunk c
                ll_part(0, c, ps1, xbts_c, h1, list(range(1, len(FBLKS))))
                epilogue(ps1[0], 2, h2[:, csl])
                epilogue(ps1[1], 3, fin[:, 1 * NBD + c * CH: 1 * NBD + (c + 1) * CH])
                if c < 7:
                    l0(c + 1, z0cc_n)
                    if c < 6:
                        z0cc_n = load_z0cc(c + 2)
                if prev_f23 is not None:
                    do_mv(c - 1, prev_f23)
                # layer 2 for chunk c; block 0 first, then next chunk's L1 head
                ps2 = new_ps(f"ps2_{c}")
                ll_part(1, c, ps2, xbts_c, h2, [0])
                if c < 7:
                    xbt0_n = load_xb(c + 1, 0)
                    ps1_n = new_ps(f"ps1_{c + 1}")
                    ll_part(0, c + 1, ps1_n, [xbt0_n], h1, [0])
                ll_part(1, c, ps2, xbts_c, h2, list(range(1, len(FBLKS))))
                f23 = {}
                for hf in range(2):
                    f23t = f23p.tile([128, CH], dt.bfloat16, tag="f23",
                                     name=f"f23_{c}_{hf}")
                    epilogue(ps2[hf], 4 + hf, f23t[:])
                    f23[hf] = f23t
                prev_f23 = f23
                if c < 7:
                    xbts_c = [xbt0_n] + [load_xb(c + 1, bi)
                                         for bi in range(1, len(FBLKS))]
                    ps1 = ps1_n
            do_mv(7, prev_f23)


    nc.compile()
    return nc


def _prep_inputs(x, W0, b0, W1, b1, W2, b2, fc_w, fc_b):
    """Host-side preprocessing -> per-core input maps (numpy only)."""
    x = np.asarray(x, dtype=np.float32)
    xT = np.ascontiguousarray(x.transpose(1, 0, 2)).reshape(F, B * D)  # [39, B*D]
    xTb = xT.astype(BF16)

    # Layer-0: z0 = KhatriRao(x0,x0) is symmetric in (f,h), so fold the weights
    # over (f,h)<->(h,f) and keep only the f<=h pairs: K 1521 -> 780 (pad 896).
    fi, hi = np.triu_indices(F)                       # 780 pairs, f<=h
    z0 = (xTb[fi, :].astype(np.float32)
          * xTb[hi, :].astype(np.float32)).astype(BF16)   # [780, B*D]
    z0p = np.zeros((K0, B * D), dtype=BF16)
    z0p[:NP0] = z0

    W0m = np.asarray(W0, dtype=np.float32).reshape(F, F, O)
    W0f = W0m[fi, hi] + np.where(fi != hi, 1.0, 0.0)[:, None] * W0m[hi, fi]

    def wdev(W, kb):
        Wb = np.zeros((kb * 128, O), dtype=np.float32)
        Wb[: W.shape[0]] = np.asarray(W, dtype=np.float32)
        # [kb*128, O] -> [128(h), kb*O] with layout w[h, k, o] = W[k*128+h, o]
        return np.ascontiguousarray(
            Wb.reshape(kb, 128, O).transpose(1, 0, 2)).reshape(128, kb * O).astype(BF16)

    w0d, w1d, w2d = wdev(W0f, K0C), wdev(W1, F), wdev(W2, F)
    bia = np.stack([np.asarray(b, dtype=np.float32)[hf * 128:(hf + 1) * 128]
                    for b in (b0, b1, b2) for hf in range(2)], axis=1)
    bia = np.ascontiguousarray(bia, dtype=np.float32)            # [128, 6]
    fcwd = np.ascontiguousarray(
        np.asarray(fc_w, dtype=np.float32).reshape(4, 128).T).astype(BF16)  # [128, 4]

    in_maps = []
    for c in range(NCORES):
        sl = np.s_[:, c * NBD:(c + 1) * NBD]
        xc = np.asarray(xTb[sl])                                  # [39, 4096]
        # -> [chunk, block, partition, f_local*CH]; every partition holds the
        # same row (broadcast), block 4 zero-padded from 7 to 8 f's
        xr = np.zeros((NCH, len(FBLKS), FBLK, CH), dtype=BF16)
        xr_v = xc.reshape(F, NCH, CH).transpose(1, 0, 2)          # [8, 39, 512]
        xr.reshape(NCH, len(FBLKS) * FBLK, CH)[:, :F] = xr_v
        x0b = np.ascontiguousarray(np.broadcast_to(
            xr.reshape(NCH, len(FBLKS), 1, FBLK * CH),
            (NCH, len(FBLKS), 128, FBLK * CH)))
        # [896, 4096] -> [bd-chunk, partition, k-chunk, col] so each chunk's
        # layer-0 contraction slab is one contiguous per-partition DMA
        z0c = np.ascontiguousarray(
            z0p[sl].reshape(K0C, 128, NCH, CH).transpose(2, 1, 0, 3))
        in_maps.append({
            "z0t": z0c, "x0b": x0b, "w0": w0d, "w1": w1d, "w2": w2d,
            "bia": bia, "fcw": fcwd,
        })
    return in_maps


def kernel(x, W0, b0, W1, b1, W2, b2, fc_w, fc_b):
    global LAST_RESULT
    if "nc" not in _CACHE:
        _CACHE["nc"] = _build_program()
    nc = _CACHE["nc"]

    in_maps = _prep_inputs(x, W0, b0, W1, b1, W2, b2, fc_w, fc_b)
    trace = bool(int(os.environ.get("NN_CIN_TRACE", "0")))
    res = run_bass_kernel_spmd(nc, in_maps, core_ids=list(range(NCORES)), trace=trace)
    LAST_RESULT = res

    fcb = float(np.asarray(fc_b, dtype=np.float32).reshape(-1)[0])
    out = np.empty((B, 1), dtype=np.float32)
    for c in range(NCORES):
        a = np.asarray(res.results[c]["acc"], dtype=np.float32).reshape(BC)
        out[c * BC:(c + 1) * BC, 0] = a + fcb
    return out



# revision 22
# speedup vs baseline: 1.1703x; 1.1703x over previous
"""
Trainium2 Bass kernel for nn_CIN (xDeepFM-style Compressed Interaction Network).

Reference computation (B=1024, F=39, D=32):
    x0 = x;  h = x
    for layer i in 0..2:
        z[b,d,:] = outer(x0[b,:,d], h[b,:,d]).flatten()     # (B, D, F*Hp)
        out = relu(z @ W_i + b_i)                           # (B, D, 256) -> (B, 256, D)
        h = out[:, :128]; finals.append(out[:, 128:])       # (last layer: all 256)
    res = concat(finals, 1).sum(-1) @ fc_w + fc_b           # (B, 1)

Strategy (data-parallel over 8 cores, 128 samples each):
  Everything on-chip lives TRANSPOSED: activations as [channel, (b,d)] so that
  - the matmul contraction (f*h) is on the partition axis (weights stationary),
  - the per-sample Khatri-Rao z-formation  z^T[f*128+h, bd] = x0^T[f,bd]*h^T[h,bd]
    is a plain VectorEngine tensor_tensor multiply against a host-precomputed
    128-partition broadcast of x0^T (per f),
  - bias+relu is a per-partition ScalarEngine activation (o on partitions),
  - the final fc_w contraction is a TensorE matvec over partitions.
  Layer-0's z is a pure function of the input x, so it is precomputed on host
  (Khatri-Rao of x0 with itself, ~0.03% of total FLOPs) and streamed.
  Layers 1 and 2 are processed per 512-wide bd-chunk back to back so the
  broadcast-x0 tiles are loaded once per chunk and reused by both layers.
  All matmul inputs are bf16 (fp32 PSUM accumulation).
"""

import os
import sys

import numpy as np

for _p in ("/opt/trn_rl_repo",):
    if os.path.isdir(_p) and _p not in sys.path:
        sys.path.append(_p)

import ml_dtypes

import concourse.bass as bass
import concourse.mybir as mybir
import concourse.tile as tile
from concourse import bacc
from concourse.bass_utils import run_bass_kernel_spmd

BF16 = ml_dtypes.bfloat16

# Problem constants (hardcoded per contract).
B, F, D = 1024, 39, 32
O = 256            # per-layer conv output channels
NCORES = 8
BC = B // NCORES   # samples per core = 128
NBD = BC * D       # bd columns per core = 4096
CH = 512           # free-dim chunk width
NCH = NBD // CH    # 8 chunks
NP0 = F * (F + 1) // 2   # layer-0 folded symmetric pairs = 780
K0C = 7                  # layer-0 K chunks (780 padded to 896)
K0 = K0C * 128
FBLK = 8           # f's per x0-broadcast DMA block
FBLKS = [8, 8, 8, 8, 7]

LAST_RESULT = None  # BassKernelResults of the most recent run (for test.py)
_CACHE = {}


def _build_program():
    """Build + compile the per-core Bass/Tile program (identical on all cores)."""
    nc = bacc.Bacc("TRN2", target_bir_lowering=False, debug=False)
    dt = mybir.dt

    z0t = nc.dram_tensor("z0t", [NCH, 128, K0C, CH], dt.bfloat16,
                         kind="ExternalInput").ap()
    # broadcast-x0, pre-arranged per (bd-chunk, f-block) so every DMA reads
    # partition-contiguous 8KB runs (descriptor-light on the HWDGE ring)
    x0b = nc.dram_tensor("x0b", [NCH, len(FBLKS), 128, FBLK * CH], dt.bfloat16,
                         kind="ExternalInput").ap()
    w0 = nc.dram_tensor("w0", [128, K0C * O], dt.bfloat16, kind="ExternalInput").ap()
    w1 = nc.dram_tensor("w1", [128, F * O], dt.bfloat16, kind="ExternalInput").ap()
    w2 = nc.dram_tensor("w2", [128, F * O], dt.bfloat16, kind="ExternalInput").ap()
    bia = nc.dram_tensor("bia", [128, 6], dt.float32, kind="ExternalInput").ap()
    fcw = nc.dram_tensor("fcw", [128, 4], dt.bfloat16, kind="ExternalInput").ap()
    acc = nc.dram_tensor("acc", [NCH, 16], dt.float32, kind="ExternalOutput").ap()

    with tile.TileContext(nc) as tc:
        with (
            tc.tile_pool(name="consts", bufs=1) as consts,
            tc.tile_pool(name="state", bufs=1) as state,
            tc.tile_pool(name="z0p", bufs=K0C) as z0p,
            tc.tile_pool(name="xbp", bufs=7) as xbp,
            tc.tile_pool(name="zp", bufs=4) as zp,
            tc.tile_pool(name="accp", bufs=2) as accp,
            tc.tile_pool(name="f23p", bufs=4) as f23p,
            tc.tile_pool(name="redp", bufs=2) as redp,
            tc.tile_pool(name="psp", bufs=8, space="PSUM") as psp,
        ):
            w0_sb = consts.tile([128, K0C * O], dt.bfloat16)
            w1_sb = consts.tile([128, F * O], dt.bfloat16)
            w2_sb = consts.tile([128, F * O], dt.bfloat16)
            bia_sb = consts.tile([128, 6], dt.float32)
            fcw_sb = consts.tile([128, 4], dt.bfloat16)

            # HAM warm-up: garbage matmuls issued before any DMA dependency so
            # the PE's activity window flips to full clock while the ~7us NEFF
            # preamble + first input DMAs are still in flight.
            warm = consts.tile([128, CH], dt.bfloat16)
            nc.vector.memset(warm[:], 0)
            warm_ps = psp.tile([128, CH], dt.float32, name="warm_ps", tag="ps")
            for _ in range(12):
                nc.tensor.matmul(warm_ps[:], lhsT=warm[:, 0:128], rhs=warm[:],
                                 start=True, stop=True)

            def wblock(w_sb, w_dram, blk):
                f0 = sum(FBLKS[:blk])
                w = FBLKS[blk]
                nc.sync.dma_start(w_sb[:, f0 * O:(f0 + w) * O],
                                  w_dram[:, f0 * O:(f0 + w) * O])

            h1 = state.tile([128, NBD], dt.bfloat16)   # layer-1 carried state^T
            h2 = state.tile([128, NBD], dt.bfloat16)   # layer-2 carried state^T
            fin = state.tile([128, 2 * NBD], dt.bfloat16)  # finals^T of layers 0,1

            def epilogue(ps, bias_col, dst):
                # dst = bf16(relu(psum + bias)), bias per-partition
                nc.scalar.activation(
                    dst, ps[:], mybir.ActivationFunctionType.Relu,
                    bias=bia_sb[:, bias_col:bias_col + 1], scale=1.0,
                )

            # ---- Layer 0, column-chunked -------------------------------------
            # z0 is the symmetric Khatri-Rao of x0 with itself, host-folded
            # over (f,h)<->(h,f) so K shrinks 1521 -> 780. It ships per
            # bd-chunk as [128p, 7k, 512] (1MB just-in-time DMAs) and L0-(c+1)
            # runs inside chunk c's pipeline as the PE gap-filler while the
            # relu(h2) -> z2 chain resolves — so the DMA head shrinks from
            # ~13MB to ~3MB before the first real matmul.
            K6 = NP0 - 6 * 128  # real contraction rows in the last k-chunk (12)

            def load_z0cc(c):
                z0_t = z0p.tile([128, K0C, CH], dt.bfloat16, name=f"z0_{c}",
                                tag="z0")
                nc.sync.dma_start(z0_t[:, 0:K0C - 1, :], z0t[c][:, 0:K0C - 1, :])
                nc.sync.dma_start(z0_t[0:K6, K0C - 1, :],
                                  z0t[c][0:K6, K0C - 1, :])
                return z0_t

            def l0(c, z0_t):
                ps0 = new_ps(f"ps0_{c}")
                for k in range(K0C):
                    np_k = K6 if k == K0C - 1 else 128
                    for hf in range(2):
                        nc.tensor.matmul(
                            ps0[hf][:],
                            lhsT=w0_sb[0:np_k,
                                       k * O + hf * 128: k * O + hf * 128 + 128],
                            rhs=z0_t[0:np_k, k, :],
                            start=(k == 0), stop=(k == K0C - 1),
                        )
                epilogue(ps0[0], 0, h1[:, c * CH: (c + 1) * CH])
                epilogue(ps0[1], 1,
                         fin[:, 0 * NBD + c * CH: 0 * NBD + (c + 1) * CH])

            # ---- Layers 1+2 fused per bd-chunk ------------------------------
            def blocks(blk_list):
                # (blk, j0, nf, f0) subgroups: 4-f DVE batches within f-blocks
                out = []
                for blk in blk_list:
                    bw = FBLKS[blk]
                    for j0, nf in ((0, 4), (4, bw - 4)):
                        out.append((blk, j0, nf, blk * FBLK + j0))
                return out

            def ll_part(li, c, ps, xbts, h_in, blk_list):
                # TT z-formation + matmuls of layer li+1 for chunk c, f-blocks
                # blk_list. One DVE op forms z^T for nf f's; the h-chunk
                # operand is re-read via a stride-0 AP dim.
                w_sb = (w1_sb, w2_sb)[li]
                csl = slice(c * CH, (c + 1) * CH)
                for blk, j0, nf, f0 in blocks(blk_list):
                    zt = zp.tile([128, 4 * CH], dt.bfloat16, tag="zt")
                    nc.vector.tensor_mul(
                        zt[:].rearrange("p (f c) -> p f c", f=4)[:, :nf],
                        xbts[blk][:, j0 * CH: (j0 + nf) * CH]
                            .rearrange("p (f c) -> p f c", f=nf),
                        h_in[:, csl].unsqueeze(1).broadcast_to((128, nf, CH)),
                    )
                    for i in range(nf):
                        f = f0 + i
                        for hf in range(2):
                            nc.tensor.matmul(
                                ps[hf][:],
                                lhsT=w_sb[:, f * O + hf * 128: f * O + hf * 128 + 128],
                                rhs=zt[:, i * CH: (i + 1) * CH],
                                start=(f == 0), stop=(f == F - 1),
                            )

            def load_xb(c, bi, eng=None):
                # two pieces aligned with ll_part's (0,4)/(4,..) DVE subgroups
                # so the first z-formation can start when half the block lands
                w = FBLKS[bi]
                xbt = xbp.tile([128, w * CH], dt.bfloat16, tag="xbt",
                               name=f"xbt_{c}_{bi}")
                e = eng or nc.sync
                e.dma_start(xbt[:, 0:4 * CH], x0b[c, bi, :, 0:4 * CH])
                e.dma_start(xbt[:, 4 * CH:w * CH], x0b[c, bi, :, 4 * CH:w * CH])
                return xbt

            def new_ps(nm):
                return [psp.tile([128, CH], dt.float32, name=f"{nm}_{hf}", tag="ps")
                        for hf in range(2)]

            def do_mv(c, f23):
                # d-sum each of the 4 finals on the DVE (columns are b-major,
                # d-minor: 16 groups of 32), then a tiny N=16 fc contraction
                # over channel partitions. Moves ~7us of N=512 matvecs off the
                # PE and shortens the serial tail after the last chunk.
                red = redp.tile([128, 4, 16], dt.bfloat16, tag="red",
                                name=f"red_{c}")
                fin4 = (fin[:].rearrange("p (j n) -> p j n", j=2)
                        [:, :, c * CH:(c + 1) * CH]
                        .rearrange("p j (b d) -> p j b d", b=16))
                with nc.allow_low_precision(
                        reason="DVE accumulates fp32 internally; bf16 is only "
                               "the stored d-sum, matching the baseline's "
                               "bf16-finals matvec precision"):
                    nc.vector.reduce_sum(red[:, 0:2, :].unsqueeze(3), fin4,
                                         axis=mybir.AxisListType.X)
                    for hf in range(2):
                        nc.vector.reduce_sum(
                            red[:, 2 + hf, :].unsqueeze(2),
                            f23[hf][:].rearrange("p (b d) -> p b d", b=16),
                            axis=mybir.AxisListType.X)
                mv = psp.tile([1, 16], dt.float32, name=f"mv_{c}", tag="ps")
                for j in range(4):
                    nc.tensor.matmul(
                        mv[:], lhsT=fcw_sb[:, j:j + 1], rhs=red[:, j, :],
                        start=(j == 0), stop=(j == 3),
                    )
                acct = accp.tile([1, 16], dt.float32, tag="acct",
                                 name=f"acct_{c}")
                nc.scalar.copy(acct[:], mv[:])
                nc.sync.dma_start(acc[c:c + 1, :], acct[:])

            # Software-pipelined main loop. PE stream per chunk c:
            #   [L1-c rest] [L0-(c+1)] [mv c-1] [L2-c blk0] [L1-head c+1]
            #   [L2-c rest]
            # L0-(c+1) has no on-chip dependency chain, so it fills the PE
            # while chunk c's relu(h2) -> z2 DVE chain resolves; by the time
            # L1-head-(c+1) issues, h1[c+1] from L0-(c+1)'s epilogue is ready.
            # head DMAs split across both rings (each gets ~half the HBM
            # share), ordered by first-use time:
            #   sync:   z0cc0 | bia | xb(0,1..4) | w2b0 | z0cc1 | fcw
            #   scalar: w0 | xb(0,0) | w1b0
            z0cc = load_z0cc(0)
            nc.sync.dma_start(bia_sb[:], bia)
            nc.scalar.dma_start(w0_sb[:], w0)
            xbts_c = [load_xb(0, 0, eng=nc.scalar)]
            nc.scalar.dma_start(w1_sb[:, 0:FBLKS[0] * O], w1[:, 0:FBLKS[0] * O])
            xbts_c += [load_xb(0, bi) for bi in range(1, len(FBLKS))]
            wblock(w2_sb, w2, 0)
            z0cc_n = load_z0cc(1)
            nc.sync.dma_start(fcw_sb[:], fcw)
            l0(0, z0cc)
            ps1 = new_ps("ps1_0")
            ll_part(0, 0, ps1, xbts_c, h1, [0])
            # bulk weights ride the Scalar ring; issued after l0(0)'s
            # epilogues so their descriptor-gen can't delay the first relus
            for blk in range(1, len(FBLKS)):
                f0 = sum(FBLKS[:blk])
                bw = FBLKS[blk]
                nc.scalar.dma_start(w1_sb[:, f0 * O:(f0 + bw) * O],
                                    w1[:, f0 * O:(f0 + bw) * O])
                nc.scalar.dma_start(w2_sb[:, f0 * O:(f0 + bw) * O],
                                    w2[:, f0 * O:(f0 + bw) * O])
            prev_f23 = None
            for c in range(8):
                csl = slice(c * CH, (c + 1) * CH)
                # rest of layer 1 for chunk c
                ll_part(0, c, ps1, xbts_c, h1, list(range(1, len(FBLKS))))
                epilogue(ps1[0], 2, h2[:, csl])
                epilogue(ps1[1], 3, fin[:, 1 * NBD + c * CH: 1 * NBD + (c + 1) * CH])
                if c < 7:
                    l0(c + 1, z0cc_n)
                    if c < 6:
                        z0cc_n = load_z0cc(c + 2)
                # layer 2 for chunk c; block 0 first, then next chunk's L1 head
                ps2 = new_ps(f"ps2_{c}")
                ll_part(1, c, ps2, xbts_c, h2, [0])
                if c < 7:
                    xbt0_n = load_xb(c + 1, 0)
                    ps1_n = new_ps(f"ps1_{c + 1}")
                    ll_part(0, c + 1, ps1_n, [xbt0_n], h1, [0])
                # mv after the critical z-formations so its DVE reduces don't
                # delay the L2/L1-head zt ops in the Vector queue
                if prev_f23 is not None:
                    do_mv(c - 1, prev_f23)
                ll_part(1, c, ps2, xbts_c, h2, list(range(1, len(FBLKS))))
                f23 = {}
                for hf in range(2):
                    f23t = f23p.tile([128, CH], dt.bfloat16, tag="f23",
                                     name=f"f23_{c}_{hf}")
                    epilogue(ps2[hf], 4 + hf, f23t[:])
                    f23[hf] = f23t
                prev_f23 = f23
                if c < 7:
                    xbts_c = [xbt0_n] + [load_xb(c + 1, bi)
                                         for bi in range(1, len(FBLKS))]
                    ps1 = ps1_n
            do_mv(7, prev_f23)


    nc.compile()
    return nc


def _prep_inputs(x, W0, b0, W1, b1, W2, b2, fc_w, fc_b):
    """Host-side preprocessing -> per-core input maps (numpy only)."""
    x = np.asarray(x, dtype=np.float32)
    xT = np.ascontiguousarray(x.transpose(1, 0, 2)).reshape(F, B * D)  # [39, B*D]
    xTb = xT.astype(BF16)

    # Layer-0: z0 = KhatriRao(x0,x0) is symmetric in (f,h), so fold the weights
    # over (f,h)<->(h,f) and keep only the f<=h pairs: K 1521 -> 780 (pad 896).
    fi, hi = np.triu_indices(F)                       # 780 pairs, f<=h
    z0 = (xTb[fi, :].astype(np.float32)
          * xTb[hi, :].astype(np.float32)).astype(BF16)   # [780, B*D]
    z0p = np.zeros((K0, B * D), dtype=BF16)
    z0p[:NP0] = z0

    W0m = np.asarray(W0, dtype=np.float32).reshape(F, F, O)
    W0f = W0m[fi, hi] + np.where(fi != hi, 1.0, 0.0)[:, None] * W0m[hi, fi]

    def wdev(W, kb):
        Wb = np.zeros((kb * 128, O), dtype=np.float32)
        Wb[: W.shape[0]] = np.asarray(W, dtype=np.float32)
        # [kb*128, O] -> [128(h), kb*O] with layout w[h, k, o] = W[k*128+h, o]
        return np.ascontiguousarray(
            Wb.reshape(kb, 128, O).transpose(1, 0, 2)).reshape(128, kb * O).astype(BF16)

    w0d, w1d, w2d = wdev(W0f, K0C), wdev(W1, F), wdev(W2, F)
    bia = np.stack([np.asarray(b, dtype=np.float32)[hf * 128:(hf + 1) * 128]
                    for b in (b0, b1, b2) for hf in range(2)], axis=1)
    bia = np.ascontiguousarray(bia, dtype=np.float32)            # [128, 6]
    fcwd = np.ascontiguousarray(
        np.asarray(fc_w, dtype=np.float32).reshape(4, 128).T).astype(BF16)  # [128, 4]

    in_maps = []
    for c in range(NCORES):
        sl = np.s_[:, c * NBD:(c + 1) * NBD]
        xc = np.asarray(xTb[sl])                                  # [39, 4096]
        # -> [chunk, block, partition, f_local*CH]; every partition holds the
        # same row (broadcast), block 4 zero-padded from 7 to 8 f's
        xr = np.zeros((NCH, len(FBLKS), FBLK, CH), dtype=BF16)
        xr_v = xc.reshape(F, NCH, CH).transpose(1, 0, 2)          # [8, 39, 512]
        xr.reshape(NCH, len(FBLKS) * FBLK, CH)[:, :F] = xr_v
        x0b = np.ascontiguousarray(np.broadcast_to(
            xr.reshape(NCH, len(FBLKS), 1, FBLK * CH),
            (NCH, len(FBLKS), 128, FBLK * CH)))
        # [896, 4096] -> [bd-chunk, partition, k-chunk, col] so each chunk's
        # layer-0 contraction slab is one contiguous per-partition DMA
        z0c = np.ascontiguousarray(
            z0p[sl].reshape(K0C, 128, NCH, CH).transpose(2, 1, 0, 3))
        in_maps.append({
            "z0t": z0c, "x0b": x0b, "w0": w0d, "w1": w1d, "w2": w2d,
            "bia": bia, "fcw": fcwd,
        })
    return in_maps


def kernel(x, W0, b0, W1, b1, W2, b2, fc_w, fc_b):
    global LAST_RESULT
    if "nc" not in _CACHE:
        _CACHE["nc"] = _build_program()
    nc = _CACHE["nc"]

    in_maps = _prep_inputs(x, W0, b0, W1, b1, W2, b2, fc_w, fc_b)
    trace = bool(int(os.environ.get("NN_CIN_TRACE", "0")))
    res = run_bass_kernel_spmd(nc, in_maps, core_ids=list(range(NCORES)), trace=trace)
    LAST_RESULT = res

    fcb = float(np.asarray(fc_b, dtype=np.float32).reshape(-1)[0])
    out = np.empty((B, 1), dtype=np.float32)
    for c in range(NCORES):
        a = np.asarray(res.results[c]["acc"], dtype=np.float32).reshape(BC)
        out[c * BC:(c + 1) * BC, 0] = a + fcb
    return out

